# revision 41
# baseline (speedup 1.0000x reference)
"""Multi-head attention (RoPE) Trainium2 Bass kernel.

Problem: B=2, S=2048, d_model=1024, 16 heads x head_dim 64, fp32.

The reference faithfully replicates a torch rank-5 reshape bug: the
attention output [1,H,B,S,D] is transposed to (0,2,1,3,4) and
flat-reshaped to [B,S,H*D] BEFORE the Wo projection. Net semantics:
  out[b2, s2, :] = flatten(O[b, h, s0:s0+16, :]) @ Wo + bo
  with h = b2*8 + s2//256, b = (s2//128)%2, s0 = (s2%128)*16,
so the projection is PER-HEAD (contraction mixes 16 seq x 64 dims of one
head) and every (b,h) yields an independent [128, 1024] output block.

Sharding (8 cores): batch (2) x head groups (4 groups of 4 heads).
Per core: QKV slices via f32r matmuls in transposed layout, RoPE
(rotate-half via a signed permutation matmul), per-head attention with
unnormalized softmax (ones-column appended to V gives the denominator),
normalize into ot64 [64, 4head, S], then per-head scrambled projection
against full Wo. Host places the 32 independent blocks and adds bo.

Scheduling notes (v2):
 - weight/x DMAs are chunked and spread over 4 queues so the first
   matmul starts ~4us in instead of ~20us.
 - softmax normalization: reciprocal_approx_fast on the denominator row
   + a tiny K=2 matmul that broadcasts both heads' 1/denom rows across
   64 partitions (replaces a DRAM round-trip partition_broadcast and a
   16x slower vector.reciprocal).
 - phase D contracts 128-deep: Wo rows for seq pair (2t, 2t+1) live in
   partitions 0-63 / 64-127; the duplicate copy of O^T in partitions
   64-127 is written shifted by one seq position so a single rhs AP
   covers both contraction halves.
 - phase D is split by head pair: D(hc=0) is emitted interleaved into
   phase C's hc=1 tiles (the PE has slack there; phase C is paced by the
   scalar engine's exp), leaving only D(hc=1) as the serial tail.
"""

import numpy as np

import concourse.bass as bass
import concourse.tile as tile
from concourse import bacc, mybir
from concourse import bass_utils

F32 = mybir.dt.float32
MM_DT = mybir.dt.float32r  # matmul operand dtype (float32r: 1 cyc/row)

B, S, DM, H, HD = 2, 2048, 1024, 16, 64
N_CORES = 8
HG = 4          # head groups (tensor-parallel factor)
GD = DM // HG   # qkv dims per core = 256
NKC = DM // 128   # d_model contraction chunks = 8
NST = S // 512    # seq tiles of 512 = 4
NSK = S // 128    # seq_k chunks of 128 = 16


def _emit(nc, tc, ap, debug=False):
    import contextlib

    ctx = contextlib.ExitStack()
    with ctx:
        consts = ctx.enter_context(tc.tile_pool(name="consts", bufs=1))
        bigp = ctx.enter_context(tc.tile_pool(name="big", bufs=1))

        # ---- persistent tiles ----
        # ones2[hi, m] = 1 where m//64 == hi: K=2 matmul broadcasts the two
        # 1/denom rows across partition halves.
        ones2 = consts.tile([2, 128], MM_DT)
        nc.gpsimd.dma_start(ones2, ap["ones2"].bitcast(MM_DT))

        # qe/ke split per (mc=head-pair, st) for fine-grained deps
        qe_t = [
            [bigp.tile([128, 512], MM_DT, name=f"qe{mc}_{st}") for st in range(NST)]
            for mc in range(2)
        ]
        ke_t = [
            [bigp.tile([128, 512], MM_DT, name=f"ke{mc}_{st}") for st in range(NST)]
            for mc in range(2)
        ]
        # Zero-padded ke per head: 64-partition-contraction f32r matmuls
        # stream at HALF rate on TRN2, so scores use full-128 contraction with
        # the other head's partitions zeroed. ke_z[hc][st][:, hi, :] holds
        # head hi's rows live, the other 64 partitions zero.
        ke_z = [
            [bigp.tile([128, 2, 512], MM_DT, name=f"kez{hc}_{st}") for st in range(NST)]
            for hc in range(2)
        ]
        for hc in range(2):
            for st in range(NST):
                nc.vector.memset(ke_z[hc][st][64:128, 0, :].bitcast(F32), 0.0)
                nc.vector.memset(ke_z[hc][st][0:64, 1, :].bitcast(F32), 0.0)
        # V natural layout + ones column: [128 seq, kc, head, 65]
        vsb = bigp.tile([128, NSK, 4, 65], MM_DT, name="vsb", tag="vsb")
        nc.vector.memset(vsb[:, :, :, 64:65].bitcast(F32), 1.0)
        # normalized attention output, heads on the free axis: [128, head, S].
        # Partitions 0-63 hold O^T; 64-127 hold a copy SHIFTED BY ONE seq
        # position (dup[64+d, h, s] = O^T[d, h, s+1]) so phase D can contract
        # seq pairs (2t, 2t+1) 128-deep with a single rhs AP.
        ot64 = bigp.tile([128, 4, S], MM_DT, name="ot64", tag="ot64")

        with tc.tile_pool(name="bconsts", bufs=1) as bconsts:
            # ---- weights to SBUF, chunked + spread across queues ----
            wq = bconsts.tile([128, NKC, GD], MM_DT)
            wqr = ap["wq"].rearrange("(kc p) m -> p kc m", p=128).bitcast(MM_DT)
            wk = bconsts.tile([128, NKC, GD], MM_DT)
            wkr = ap["wk"].rearrange("(kc p) m -> p kc m", p=128).bitcast(MM_DT)
            wv = bconsts.tile([128, NKC, GD], MM_DT)
            wvr = ap["wv"].rearrange("(kc p) m -> p kc m", p=128).bitcast(MM_DT)
            rot = bconsts.tile([128, 128], MM_DT)
            nc.gpsimd.dma_start(rot, ap["rot"].bitcast(MM_DT))
            bqc = bconsts.tile([128, 2], F32)
            nc.gpsimd.dma_start(bqc, ap["bq2"].rearrange("c p -> p c"))
            bkc = bconsts.tile([128, 2], F32)
            nc.gpsimd.dma_start(bkc, ap["bk2"].rearrange("c p -> p c"))
            bvb = bconsts.tile([128, GD], F32)
            nc.gpsimd.dma_start(bvb, ap["bv"].partition_broadcast(128))
            # critical-first: small leading chunks of wq/wk so kc0's matmuls
            # start ASAP; the rest as few big DMAs (each dma_start costs
            # ~650ns of queue issue time)
            nc.sync.dma_start(wq[:, 0:2, :], wqr[:, 0:2, :])
            nc.sync.dma_start(wk[:, 0:2, :], wkr[:, 0:2, :])
            nc.sync.dma_start(wq[:, 2:8, :], wqr[:, 2:8, :])
            nc.sync.dma_start(wk[:, 2:8, :], wkr[:, 2:8, :])
            # RoPE tables: DMAs deferred until after st0's matmul emission
            cosb = bconsts.tile([128, S], F32)
            sinb = bconsts.tile([128, S], F32)

            # ================= Phase B: QKV projections + RoPE =============
            xtr = ap["xt"].rearrange("(k2 p2 p) s -> p k2 p2 s", p=128, p2=2)
            with (
                tc.tile_pool(name="xt", bufs=6) as xt_pool,
                tc.tile_pool(name="raw", bufs=1) as raw_pool,
                tc.tile_pool(name="t1", bufs=2) as t1_pool,
                tc.tile_pool(name="ps_qk", bufs=1, space="PSUM") as ps_qk,
                tc.tile_pool(name="ps_v", bufs=1, space="PSUM") as ps_v,
            ):
                for st in range(NST):
                    sl = slice(st * 512, (st + 1) * 512)
                    pqk = {}
                    pv = {}
                    for tgt in range(2):
                        for mc in range(2):
                            pqk[tgt, mc] = ps_qk.tile(
                                [128, 512], F32, name=f"pqk{tgt}{mc}", tag=f"qk{tgt}{mc}"
                            )
                    for ss in range(4):
                        pv[ss] = ps_v.tile([128, GD], F32, name=f"pv{ss}", tag=f"v{ss}")
                    xts = []
                    for kc2 in range(NKC // 2):
                        xt2 = xt_pool.tile([128, 2, 512], MM_DT)
                        xts.append(xt2)
                        q = [nc.scalar, nc.gpsimd][kc2 % 2]
                        q.dma_start(xt2, xtr[:, kc2, :, sl].bitcast(MM_DT))
                        if st == 0 and kc2 == 0:
                            # wv needed only after the whole QK block
                            nc.scalar.dma_start(wv[:, 0:2, :], wvr[:, 0:2, :])
                        if st == 0 and kc2 == 2:
                            nc.scalar.dma_start(wv[:, 2:8, :], wvr[:, 2:8, :])
                        for j in range(2):
                            kc = kc2 * 2 + j
                            for tgt in range(2):
                                w_sb = wq if tgt == 0 else wk
                                for mc in range(2):
                                    nc.tensor.matmul(
                                        pqk[tgt, mc],
                                        lhsT=w_sb[:, kc, mc * 128:(mc + 1) * 128],
                                        rhs=xt2[:, j, :],
                                        start=(kc == 0),
                                        stop=(kc == NKC - 1),
                                    )
                    # V after the full QK block: decouples from the wv DMA
                    # and from pv-bank availability at tile start
                    for kc2 in range(NKC // 2):
                        for j in range(2):
                            kc = kc2 * 2 + j
                            for ss in range(4):
                                nc.tensor.matmul(
                                    pv[ss],
                                    lhsT=xts[kc2][:, j, ss * 128:(ss + 1) * 128],
                                    rhs=wv[:, kc, :],
                                    start=(kc == 0),
                                    stop=(kc == NKC - 1),
                                )
                    if st == 0:
                        # RoPE tables: needed at first rope writeback (~21us)
                        nc.scalar.dma_start(cosb, ap["cosb"])
                        nc.gpsimd.dma_start(sinb, ap["sinb"])
                    # issue all accumulator drains first: the QK psum slots gate
                    # the next seq tile's matmuls, so don't interleave the slower
                    # RoPE chain between them
                    raws = {}
                    for tgt in range(2):
                        bias = bqc if tgt == 0 else bkc
                        for mc in range(2):
                            raw = raw_pool.tile(
                                [128, 512], MM_DT, name=f"raw{tgt}{mc}", tag=f"raw{tgt}{mc}"
                            )
                            nc.vector.tensor_scalar_add(raw, pqk[tgt, mc], bias[:, mc:mc + 1])
                            raws[tgt, mc] = raw
                    for ss in range(4):
                        nc.vector.tensor_add(
                            vsb[:, st * 4 + ss, :, 0:64],
                            pv[ss].rearrange("p (h d) -> p h d", h=4),
                            bvb.rearrange("p (h d) -> p h d", h=4),
                        )
                    rpss = {}
                    for tgt in (1, 0):  # K first: phase C needs ke_z earliest
                        for mc in range(2):
                            rps = ps_v.tile(
                                [128, 512], F32, name=f"rps{tgt}{mc}", tag=f"v{tgt * 2 + mc}"
                            )
                            nc.tensor.matmul(rps, lhsT=rot, rhs=raws[tgt, mc], start=True, stop=True)
                            rpss[tgt, mc] = rps
                    for tgt in (1, 0):
                        dst = qe_t if tgt == 0 else ke_t
                        for mc in range(2):
                            t1 = t1_pool.tile([128, 512], F32)
                            nc.vector.tensor_mul(t1, rpss[tgt, mc], sinb[:, sl])
                            d = dst[mc][st]
                            nc.vector.tensor_mul(d, raws[tgt, mc], cosb[:, sl])
                            nc.vector.tensor_add(d, d, t1)
                            if tgt == 1:
                                # scatter ke halves into the zero-padded tiles
                                qz = nc.gpsimd if mc == 0 else nc.sync
                                qz.dma_start(ke_z[mc][st][0:64, 0, :], d[0:64, :])
                                qz.dma_start(ke_z[mc][st][64:128, 1, :], d[64:128, :])

        if debug:
            for mc in range(2):
                for st in range(NST):
                    nc.sync.dma_start(ap["qe_dbg"][mc, st], qe_t[mc][st].bitcast(F32))
                    nc.sync.dma_start(ap["ke_dbg"][mc, st], ke_t[mc][st].bitcast(F32))
            nc.sync.dma_start(ap["v_dbg"], vsb.bitcast(F32))

        # ================= Phase C: attention (+ interleaved phase D) =====
        # phase D view: ot_rp[p, h, s2r, g] = ot64[p, h, s2r*16+g]; the rhs for
        # contraction pair t reads g=2t across all 128 partitions (upper half
        # holds the shift-by-one dup = seq 2t+1 data).
        ot_rp = ot64.rearrange("p h (s2r g) -> p h s2r g", g=16)
        LAG = 2  # AV matmuls trail score matmuls by LAG kc iterations
        with (
            tc.tile_pool(name="e", bufs=LAG + 2) as e_pool,
            tc.tile_pool(name="usb", bufs=2) as usb_pool,
            tc.tile_pool(name="rcp", bufs=2) as rcp_pool,
            tc.tile_pool(name="wop", bufs=1) as wo_pool,
            tc.tile_pool(name="ysb", bufs=1) as y_pool,
            tc.tile_pool(name="ps_s", bufs=2, space="PSUM") as ps_s,
            tc.tile_pool(name="ps_u", bufs=1, space="PSUM") as ps_u,
            tc.tile_pool(name="ps_y", bufs=1, space="PSUM") as ps_y,
        ):
            # full Wo resident: [128, c(pair), mc, m]; lhsT for (mc, t) is
            # wo_t[:, t, mc, :] = Wo rows 128t..128t+127, cols mc*128..+128
            wo_t = wo_pool.tile([128, 8, NKC, 128], MM_DT, name="wo_t", tag="wo_t")
            for mc in range(NKC):
                nc.sync.dma_start(
                    wo_t[:, :, mc, :],
                    ap["wo"][:, mc * 128:(mc + 1) * 128]
                    .rearrange("(c p) m -> p c m", p=128)
                    .bitcast(MM_DT),
                )

            def emit_d_chunk(hc, mc):
                py = ps_y.tile([128, 256], F32, name=f"py{hc}{mc}", tag=f"py{mc % 2}")
                for t in range(8):
                    nc.tensor.matmul(
                        py,
                        lhsT=wo_t[:, t, mc, :],
                        rhs=ot_rp[:, hc * 2:hc * 2 + 2, :, 2 * t],
                        start=(t == 0),
                        stop=(t == 7),
                    )
                ysb = y_pool.tile([128, 256], F32, name="ysb", tag=f"ysb{mc % 2}")
                nc.vector.tensor_copy(ysb, py)
                nc.sync.dma_start(
                    ap["ypt"][mc * 128:(mc + 1) * 128, hc * 256:(hc + 1) * 256], ysb
                )

            for hc in range(2):
                for qt in range(NST):
                    qs0 = qt * 512
                    u = [
                        ps_u.tile([65, 512], F32, name=f"u{hi}", tag=f"u{hi}")
                        for hi in range(2)
                    ]
                    es = {}
                    for kc in range(NSK + LAG):
                        if kc >= LAG:
                            ka = kc - LAG
                            for hi in range(2):
                                nc.tensor.matmul(
                                    u[hi],
                                    lhsT=vsb[:, ka, hc * 2 + hi, :],
                                    rhs=es[ka][:, hi * 512:(hi + 1) * 512],
                                    start=(ka == 0),
                                    stop=(ka == NSK - 1),
                                )
                            if ka > 0:
                                del es[ka - 1]
                        if kc < NSK:
                            # both heads' scores side by side in one 2-bank group;
                            # full-128 contraction via zero-padded ke (64-deep
                            # f32r matmuls stream at half rate)
                            g = ps_s.tile([128, 1024], F32, tag="sg", name="sg")
                            for hi in range(2):
                                nc.tensor.matmul(
                                    g[:, hi * 512:(hi + 1) * 512],
                                    lhsT=ke_z[hc][kc // 4][:, hi, (kc % 4) * 128:(kc % 4 + 1) * 128],
                                    rhs=qe_t[hc][qt],
                                    start=True,
                                    stop=True,
                                )
                            e = e_pool.tile([128, 1024], MM_DT, name="e", tag="e")
                            nc.scalar.activation(
                                e, g, mybir.ActivationFunctionType.Exp, scale=0.125
                            )
                            es[kc] = e
                    # ---- normalize: ot = U[0:64] * (1/U[64]) ----
                    usbs = []
                    for hi in range(2):
                        usb = usb_pool.tile([65, 512], F32, tag=f"usb{hi}")
                        nc.vector.tensor_copy(usb, u[hi])
                        usbs.append(usb)
                    rbs = []
                    for hi in range(2):
                        # custom DVE ops mis-read inputs at partition base 64:
                        # copy the denominator row down to partition 0 first
                        d1 = rcp_pool.tile([1, 512], F32, tag=f"d{hi}")
                        nc.vector.tensor_copy(d1, usbs[hi][64:65, :])
                        r1 = rcp_pool.tile([1, 512], F32, tag=f"r{hi}")
                        nc.vector.reciprocal_approx_fast(r1, d1)
                        # gpsimd broadcast of 1/denom across 64 partitions
                        dbc = usb_pool.tile([64, 512], F32, tag=f"dbc{hi}")
                        nc.gpsimd.partition_broadcast(dbc, r1)
                        rbs.append(dbc)
                    for hi in range(2):
                        h = hc * 2 + hi
                        nc.vector.tensor_mul(
                            ot64[0:64, h, qs0:qs0 + 512], usbs[hi][0:64, :], rbs[hi]
                        )
                        # shift-by-one dup for phase D's 128-deep contraction;
                        # slot qs0+511 (g=15) is never read by phase D
                        nc.sync.dma_start(
                            ot64[64:128, h, qs0:qs0 + 511],
                            ot64[0:64, h, qs0 + 1:qs0 + 512],
                        )
                    if hc == 1:
                        # fill PE slack in the scalar-paced hc=1 tiles with
                        # the first head pair's output projection
                        emit_d_chunk(0, 2 * qt)
                        emit_d_chunk(0, 2 * qt + 1)
            for mc in range(NKC):
                emit_d_chunk(1, mc)
            if debug:
                nc.sync.dma_start(ap["ot_dbg"], ot64.bitcast(F32))


def _build(debug=False):
    nc = bacc.Bacc("TRN2", target_bir_lowering=False, debug=False, num_devices=N_CORES)
    ap = {}
    ap["xt"] = nc.dram_tensor("xt", [DM, S], F32, kind="ExternalInput").ap()
    ap["wq"] = nc.dram_tensor("wq", [DM, GD], F32, kind="ExternalInput").ap()
    ap["wk"] = nc.dram_tensor("wk", [DM, GD], F32, kind="ExternalInput").ap()
    ap["wv"] = nc.dram_tensor("wv", [DM, GD], F32, kind="ExternalInput").ap()
    ap["wo"] = nc.dram_tensor("wo", [DM, DM], F32, kind="ExternalInput").ap()
    ap["bq2"] = nc.dram_tensor("bq2", [2, 128], F32, kind="ExternalInput").ap()
    ap["bk2"] = nc.dram_tensor("bk2", [2, 128], F32, kind="ExternalInput").ap()
    ap["bv"] = nc.dram_tensor("bv", [GD], F32, kind="ExternalInput").ap()
    ap["cosb"] = nc.dram_tensor("cosb", [128, S], F32, kind="ExternalInput").ap()
    ap["sinb"] = nc.dram_tensor("sinb", [128, S], F32, kind="ExternalInput").ap()
    ap["rot"] = nc.dram_tensor("rot", [128, 128], F32, kind="ExternalInput").ap()
    ap["ones2"] = nc.dram_tensor("ones2", [2, 128], F32, kind="ExternalInput").ap()
    # per-core output: Y^T [1024, 512] (columns = 4 heads x 128 block rows)
    ap["ypt"] = nc.dram_tensor("ypt", [DM, 512], F32, kind="ExternalOutput").ap()
    if debug:
        ap["qe_dbg"] = nc.dram_tensor("qe_dbg", [2, NST, 128, 512], F32, kind="ExternalOutput").ap()
        ap["ke_dbg"] = nc.dram_tensor("ke_dbg", [2, NST, 128, 512], F32, kind="ExternalOutput").ap()
        ap["v_dbg"] = nc.dram_tensor("v_dbg", [128, NSK, 4, 65], F32, kind="ExternalOutput").ap()
        ap["ot_dbg"] = nc.dram_tensor("ot_dbg", [128, 4, S], F32, kind="ExternalOutput").ap()

    with tile.TileContext(nc) as tc:
        _emit(nc, tc, ap, debug=debug)
    nc.compile()
    return nc


_CACHE = {}


def _rope_tables():
    inv_freq = (1.0 / (10000.0 ** (np.arange(0, HD, 2, dtype=np.float32) / HD))).astype(np.float32)
    t = np.arange(S, dtype=np.float32)
    freqs = np.outer(t, inv_freq).astype(np.float32)  # [S, 32]
    emb = np.concatenate([freqs, freqs], axis=-1)  # [S, 64]
    cosT = np.cos(emb).astype(np.float32).T  # [64, S]
    sinT = np.sin(emb).astype(np.float32).T
    cosb = np.ascontiguousarray(np.concatenate([cosT, cosT], axis=0))  # [128, S]
    sinb = np.ascontiguousarray(np.concatenate([sinT, sinT], axis=0))
    return cosb, sinb


def _rot_matrix():
    p64 = np.zeros((HD, HD), dtype=np.float32)
    for i in range(32):
        p64[i, i + 32] = -1.0
        p64[i + 32, i] = 1.0
    p = np.zeros((128, 128), dtype=np.float32)
    p[0:64, 0:64] = p64
    p[64:128, 64:128] = p64
    return np.ascontiguousarray(p.T)  # lhsT = P^T


def kernel(x, Wq, bq, Wk, bk, Wv, bv, Wo, bo):
    x = np.asarray(x, dtype=np.float32)
    Wq, bq = np.asarray(Wq, np.float32), np.asarray(bq, np.float32)
    Wk, bk = np.asarray(Wk, np.float32), np.asarray(bk, np.float32)
    Wv, bv = np.asarray(Wv, np.float32), np.asarray(bv, np.float32)
    Wo, bo = np.asarray(Wo, np.float32), np.asarray(bo, np.float32)

    if "nc" not in _CACHE:
        _CACHE["nc"] = _build()
    nc = _CACHE["nc"]

    cosb, sinb = _rope_tables()
    rot = _rot_matrix()
    ones2 = np.zeros((2, 128), dtype=np.float32)
    ones2[0, 0:64] = 1.0
    ones2[1, 64:128] = 1.0
    xt_b = [np.ascontiguousarray(x[b].T) for b in range(B)]  # [DM, S]
    wo_c = np.ascontiguousarray(Wo)

    in_maps = []
    for c in range(N_CORES):
        b, hg = divmod(c, HG)
        sl = slice(hg * GD, (hg + 1) * GD)
        in_maps.append(
            {
                "xt": xt_b[b],
                "wq": np.ascontiguousarray(Wq[:, sl]),
                "wk": np.ascontiguousarray(Wk[:, sl]),
                "wv": np.ascontiguousarray(Wv[:, sl]),
                "wo": wo_c,
                "bq2": np.ascontiguousarray(bq[sl].reshape(2, 128)),
                "bk2": np.ascontiguousarray(bk[sl].reshape(2, 128)),
                "bv": np.ascontiguousarray(bv[sl]),
                "cosb": cosb,
                "sinb": sinb,
                "rot": rot,
                "ones2": ones2,
            }
        )

    res = bass_utils.run_bass_kernel_spmd(nc, in_maps, core_ids=list(range(N_CORES)))
    _CACHE["last_results"] = res

    # Block placement: core (b, hg), local head hl -> global head h = hg*4+hl,
    # lands at out[h//8, (h%8)*256 + b*128 : +128, :].
    out = np.empty((B, S, DM), dtype=np.float32)
    for c in range(N_CORES):
        b, hg = divmod(c, HG)
        ypt = res.results[c]["ypt"]  # [1024, 512]
        for hl in range(4):
            h = hg * 4 + hl
            b2 = h // 8
            s2 = (h % 8) * 256 + b * 128
            out[b2, s2:s2 + 128, :] = ypt[:, hl * 128:(hl + 1) * 128].T
    out += bo[None, None, :]
    return out


# revision 43
# speedup vs baseline: 1.0338x; 1.0338x over previous
"""Multi-head attention (RoPE) Trainium2 Bass kernel.

Problem: B=2, S=2048, d_model=1024, 16 heads x head_dim 64, fp32.

The reference faithfully replicates a torch rank-5 reshape bug: the
attention output [1,H,B,S,D] is transposed to (0,2,1,3,4) and
flat-reshaped to [B,S,H*D] BEFORE the Wo projection. Net semantics:
  out[b2, s2, :] = flatten(O[b, h, s0:s0+16, :]) @ Wo + bo
  with h = b2*8 + s2//256, b = (s2//128)%2, s0 = (s2%128)*16,
so the projection is PER-HEAD (contraction mixes 16 seq x 64 dims of one
head) and every (b,h) yields an independent [128, 1024] output block.

Sharding (8 cores): batch (2) x head groups (4 groups of 4 heads).
Per core: QKV slices via f32r matmuls in transposed layout, RoPE
(rotate-half via a signed permutation matmul), per-head attention with
unnormalized softmax (ones-column appended to V gives the denominator),
normalize into ot64 [64, 4head, S], then per-head scrambled projection
against full Wo. Host places the 32 independent blocks and adds bo.

Scheduling notes (v2):
 - weight/x DMAs are chunked and spread over 4 queues so the first
   matmul starts ~4us in instead of ~20us.
 - softmax normalization: reciprocal_approx_fast on the denominator row
   + a tiny K=2 matmul that broadcasts both heads' 1/denom rows across
   64 partitions (replaces a DRAM round-trip partition_broadcast and a
   16x slower vector.reciprocal).
 - phase D contracts 128-deep: Wo rows for seq pair (2t, 2t+1) live in
   partitions 0-63 / 64-127; the duplicate copy of O^T in partitions
   64-127 is written shifted by one seq position so a single rhs AP
   covers both contraction halves.
 - phase D is split by head pair: D(hc=0) is emitted interleaved into
   phase C's hc=1 tiles (the PE has slack there; phase C is paced by the
   scalar engine's exp), leaving only D(hc=1) as the serial tail.
"""

import numpy as np

import concourse.bass as bass
import concourse.tile as tile
from concourse import bacc, mybir
from concourse import bass_utils

F32 = mybir.dt.float32
MM_DT = mybir.dt.float32r  # matmul operand dtype (float32r: 1 cyc/row)

B, S, DM, H, HD = 2, 2048, 1024, 16, 64
N_CORES = 8
HG = 4          # head groups (tensor-parallel factor)
GD = DM // HG   # qkv dims per core = 256
NKC = DM // 128   # d_model contraction chunks = 8
NST = S // 512    # seq tiles of 512 = 4
NSK = S // 128    # seq_k chunks of 128 = 16


def _emit(nc, tc, ap, debug=False):
    import contextlib

    ctx = contextlib.ExitStack()
    with ctx:
        consts = ctx.enter_context(tc.tile_pool(name="consts", bufs=1))
        bigp = ctx.enter_context(tc.tile_pool(name="big", bufs=1))

        # ---- persistent tiles ----
        # ones2[hi, m] = 1 where m//64 == hi: K=2 matmul broadcasts the two
        # 1/denom rows across partition halves.
        ones2 = consts.tile([2, 128], MM_DT)
        nc.gpsimd.dma_start(ones2, ap["ones2"].bitcast(MM_DT))

        # qe/ke split per (mc=head-pair, st) for fine-grained deps
        qe_t = [
            [bigp.tile([128, 512], MM_DT, name=f"qe{mc}_{st}") for st in range(NST)]
            for mc in range(2)
        ]
        ke_t = [
            [bigp.tile([128, 512], MM_DT, name=f"ke{mc}_{st}") for st in range(NST)]
            for mc in range(2)
        ]
        # Zero-padded ke per head: 64-partition-contraction f32r matmuls
        # stream at HALF rate on TRN2, so scores use full-128 contraction with
        # the other head's partitions zeroed. ke_z[hc][st][:, hi, :] holds
        # head hi's rows live, the other 64 partitions zero.
        ke_z = [
            [bigp.tile([128, 2, 512], MM_DT, name=f"kez{hc}_{st}") for st in range(NST)]
            for hc in range(2)
        ]
        for hc in range(2):
            for st in range(NST):
                nc.vector.memset(ke_z[hc][st][64:128, 0, :].bitcast(F32), 0.0)
                nc.vector.memset(ke_z[hc][st][0:64, 1, :].bitcast(F32), 0.0)
        # V natural layout + ones column: [128 seq, kc, head, 65]
        vsb = bigp.tile([128, NSK, 4, 65], MM_DT, name="vsb", tag="vsb")
        nc.vector.memset(vsb[:, :, :, 64:65].bitcast(F32), 1.0)
        # normalized attention output, heads on the free axis: [128, head, S].
        # Partitions 0-63 hold O^T; 64-127 hold a copy SHIFTED BY ONE seq
        # position (dup[64+d, h, s] = O^T[d, h, s+1]) so phase D can contract
        # seq pairs (2t, 2t+1) 128-deep with a single rhs AP.
        ot64 = bigp.tile([128, 4, S], MM_DT, name="ot64", tag="ot64")

        with tc.tile_pool(name="bconsts", bufs=1) as bconsts:
            # ---- weights to SBUF, chunked + spread across queues ----
            wq = bconsts.tile([128, NKC, GD], MM_DT)
            wqr = ap["wq"].rearrange("(kc p) m -> p kc m", p=128).bitcast(MM_DT)
            wk = bconsts.tile([128, NKC, GD], MM_DT)
            wkr = ap["wk"].rearrange("(kc p) m -> p kc m", p=128).bitcast(MM_DT)
            wv = bconsts.tile([128, NKC, GD], MM_DT)
            wvr = ap["wv"].rearrange("(kc p) m -> p kc m", p=128).bitcast(MM_DT)
            rot = bconsts.tile([128, 128], MM_DT)
            nc.gpsimd.dma_start(rot, ap["rot"].bitcast(MM_DT))
            bqc = bconsts.tile([128, 2], F32)
            nc.gpsimd.dma_start(bqc, ap["bq2"].rearrange("c p -> p c"))
            bkc = bconsts.tile([128, 2], F32)
            nc.gpsimd.dma_start(bkc, ap["bk2"].rearrange("c p -> p c"))
            bvb = bconsts.tile([128, GD], F32)
            nc.gpsimd.dma_start(bvb, ap["bv"].partition_broadcast(128))
            # critical-first: small leading chunks of wq/wk so kc0's matmuls
            # start ASAP; the rest as few big DMAs (each dma_start costs
            # ~650ns of queue issue time)
            nc.sync.dma_start(wq[:, 0:2, :], wqr[:, 0:2, :])
            nc.sync.dma_start(wk[:, 0:2, :], wkr[:, 0:2, :])
            nc.sync.dma_start(wq[:, 2:8, :], wqr[:, 2:8, :])
            nc.sync.dma_start(wk[:, 2:8, :], wkr[:, 2:8, :])
            # RoPE tables: DMAs deferred until after st0's matmul emission
            cosb = bconsts.tile([128, S], F32)
            sinb = bconsts.tile([128, S], F32)

            # ================= Phase B: QKV projections + RoPE =============
            xtr = ap["xt"].rearrange("(k2 p2 p) s -> p k2 p2 s", p=128, p2=2)
            with (
                tc.tile_pool(name="xt", bufs=6) as xt_pool,
                tc.tile_pool(name="raw", bufs=1) as raw_pool,
                tc.tile_pool(name="t1", bufs=2) as t1_pool,
                tc.tile_pool(name="ps_qk", bufs=1, space="PSUM") as ps_qk,
                tc.tile_pool(name="ps_v", bufs=1, space="PSUM") as ps_v,
            ):
                for st in range(NST):
                    sl = slice(st * 512, (st + 1) * 512)
                    pqk = {}
                    pv = {}
                    for tgt in range(2):
                        for mc in range(2):
                            pqk[tgt, mc] = ps_qk.tile(
                                [128, 512], F32, name=f"pqk{tgt}{mc}", tag=f"qk{tgt}{mc}"
                            )
                    for ss in range(4):
                        pv[ss] = ps_v.tile([128, GD], F32, name=f"pv{ss}", tag=f"v{ss}")
                    xts = []
                    for kc2 in range(NKC // 2):
                        xt2 = xt_pool.tile([128, 2, 512], MM_DT)
                        xts.append(xt2)
                        if st == 0:
                            q = [nc.scalar, nc.gpsimd][kc2 % 2]
                        else:
                            q = [nc.scalar, nc.gpsimd, nc.sync][(st * 4 + kc2) % 3]
                        q.dma_start(xt2, xtr[:, kc2, :, sl].bitcast(MM_DT))
                        if st == 0 and kc2 == 0:
                            # wv needed only after the whole QK block
                            nc.scalar.dma_start(wv[:, 0:2, :], wvr[:, 0:2, :])
                            nc.sync.dma_start(wv[:, 2:8, :], wvr[:, 2:8, :])
                        for j in range(2):
                            kc = kc2 * 2 + j
                            for tgt in range(2):
                                w_sb = wq if tgt == 0 else wk
                                for mc in range(2):
                                    nc.tensor.matmul(
                                        pqk[tgt, mc],
                                        lhsT=w_sb[:, kc, mc * 128:(mc + 1) * 128],
                                        rhs=xt2[:, j, :],
                                        start=(kc == 0),
                                        stop=(kc == NKC - 1),
                                    )
                    # V after the full QK block: decouples from the wv DMA
                    # and from pv-bank availability at tile start
                    for kc2 in range(NKC // 2):
                        for j in range(2):
                            kc = kc2 * 2 + j
                            for ss in range(4):
                                nc.tensor.matmul(
                                    pv[ss],
                                    lhsT=xts[kc2][:, j, ss * 128:(ss + 1) * 128],
                                    rhs=wv[:, kc, :],
                                    start=(kc == 0),
                                    stop=(kc == NKC - 1),
                                )
                    if st == 0:
                        # RoPE tables: needed at first rope writeback (~21us)
                        nc.sync.dma_start(cosb, ap["cosb"])
                        nc.gpsimd.dma_start(sinb, ap["sinb"])
                    # issue all accumulator drains first: the QK psum slots gate
                    # the next seq tile's matmuls, so don't interleave the slower
                    # RoPE chain between them
                    raws = {}
                    for tgt in range(2):
                        bias = bqc if tgt == 0 else bkc
                        for mc in range(2):
                            raw = raw_pool.tile(
                                [128, 512], MM_DT, name=f"raw{tgt}{mc}", tag=f"raw{tgt}{mc}"
                            )
                            nc.vector.tensor_scalar_add(raw, pqk[tgt, mc], bias[:, mc:mc + 1])
                            raws[tgt, mc] = raw
                    for ss in range(4):
                        nc.vector.tensor_add(
                            vsb[:, st * 4 + ss, :, 0:64],
                            pv[ss].rearrange("p (h d) -> p h d", h=4),
                            bvb.rearrange("p (h d) -> p h d", h=4),
                        )
                    rpss = {}
                    for tgt in (1, 0):  # K first: phase C needs ke_z earliest
                        for mc in range(2):
                            rps = ps_v.tile(
                                [128, 512], F32, name=f"rps{tgt}{mc}", tag=f"v{tgt * 2 + mc}"
                            )
                            nc.tensor.matmul(rps, lhsT=rot, rhs=raws[tgt, mc], start=True, stop=True)
                            rpss[tgt, mc] = rps
                    for tgt in (1, 0):
                        dst = qe_t if tgt == 0 else ke_t
                        for mc in range(2):
                            t1 = t1_pool.tile([128, 512], F32)
                            nc.vector.tensor_mul(t1, rpss[tgt, mc], sinb[:, sl])
                            d = dst[mc][st]
                            nc.vector.tensor_mul(d, raws[tgt, mc], cosb[:, sl])
                            nc.vector.tensor_add(d, d, t1)
                            if tgt == 1:
                                # scatter ke halves into the zero-padded tiles
                                qz = nc.gpsimd if mc == 0 else nc.sync
                                qz.dma_start(ke_z[mc][st][0:64, 0, :], d[0:64, :])
                                qz.dma_start(ke_z[mc][st][64:128, 1, :], d[64:128, :])

        if debug:
            for mc in range(2):
                for st in range(NST):
                    nc.sync.dma_start(ap["qe_dbg"][mc, st], qe_t[mc][st].bitcast(F32))
                    nc.sync.dma_start(ap["ke_dbg"][mc, st], ke_t[mc][st].bitcast(F32))
            nc.sync.dma_start(ap["v_dbg"], vsb.bitcast(F32))

        # ================= Phase C: attention (+ interleaved phase D) =====
        # phase D view: ot_rp[p, h, s2r, g] = ot64[p, h, s2r*16+g]; the rhs for
        # contraction pair t reads g=2t across all 128 partitions (upper half
        # holds the shift-by-one dup = seq 2t+1 data).
        ot_rp = ot64.rearrange("p h (s2r g) -> p h s2r g", g=16)
        LAG = 2  # AV matmuls trail score matmuls by LAG kc iterations
        with (
            tc.tile_pool(name="e", bufs=LAG + 2) as e_pool,
            tc.tile_pool(name="usb", bufs=2) as usb_pool,
            tc.tile_pool(name="rcp", bufs=2) as rcp_pool,
            tc.tile_pool(name="wop", bufs=1) as wo_pool,
            tc.tile_pool(name="ysb", bufs=1) as y_pool,
            tc.tile_pool(name="ps_s", bufs=2, space="PSUM") as ps_s,
            tc.tile_pool(name="ps_u", bufs=1, space="PSUM") as ps_u,
            tc.tile_pool(name="ps_y", bufs=1, space="PSUM") as ps_y,
        ):
            # full Wo resident: [128, c(pair), mc, m]; lhsT for (mc, t) is
            # wo_t[:, t, mc, :] = Wo rows 128t..128t+127, cols mc*128..+128
            wo_t = wo_pool.tile([128, 8, NKC, 128], MM_DT, name="wo_t", tag="wo_t")
            for mc in range(NKC):
                nc.sync.dma_start(
                    wo_t[:, :, mc, :],
                    ap["wo"][:, mc * 128:(mc + 1) * 128]
                    .rearrange("(c p) m -> p c m", p=128)
                    .bitcast(MM_DT),
                )

            def emit_d_chunk(hc, mc):
                py = ps_y.tile([128, 256], F32, name=f"py{hc}{mc}", tag=f"py{mc % 2}")
                for t in range(8):
                    nc.tensor.matmul(
                        py,
                        lhsT=wo_t[:, t, mc, :],
                        rhs=ot_rp[:, hc * 2:hc * 2 + 2, :, 2 * t],
                        start=(t == 0),
                        stop=(t == 7),
                    )
                ysb = y_pool.tile([128, 256], F32, name="ysb", tag=f"ysb{mc % 2}")
                nc.vector.tensor_copy(ysb, py)
                nc.sync.dma_start(
                    ap["ypt"][mc * 128:(mc + 1) * 128, hc * 256:(hc + 1) * 256], ysb
                )

            for hc in range(2):
                for qt in range(NST):
                    qs0 = qt * 512
                    u = [
                        ps_u.tile([65, 512], F32, name=f"u{hi}", tag=f"u{hi}")
                        for hi in range(2)
                    ]
                    es = {}
                    for kc in range(NSK + LAG):
                        if kc >= LAG:
                            ka = kc - LAG
                            for hi in range(2):
                                nc.tensor.matmul(
                                    u[hi],
                                    lhsT=vsb[:, ka, hc * 2 + hi, :],
                                    rhs=es[ka][:, hi * 512:(hi + 1) * 512],
                                    start=(ka == 0),
                                    stop=(ka == NSK - 1),
                                )
                            if ka > 0:
                                del es[ka - 1]
                        if kc < NSK:
                            # both heads' scores side by side in one 2-bank group;
                            # full-128 contraction via zero-padded ke (64-deep
                            # f32r matmuls stream at half rate)
                            g = ps_s.tile([128, 1024], F32, tag="sg", name="sg")
                            for hi in range(2):
                                nc.tensor.matmul(
                                    g[:, hi * 512:(hi + 1) * 512],
                                    lhsT=ke_z[hc][kc // 4][:, hi, (kc % 4) * 128:(kc % 4 + 1) * 128],
                                    rhs=qe_t[hc][qt],
                                    start=True,
                                    stop=True,
                                )
                            e = e_pool.tile([128, 1024], MM_DT, name="e", tag="e")
                            nc.scalar.activation(
                                e, g, mybir.ActivationFunctionType.Exp, scale=0.125
                            )
                            es[kc] = e
                    # ---- normalize: ot = U[0:64] * (1/U[64]) ----
                    usbs = []
                    for hi in range(2):
                        usb = usb_pool.tile([65, 512], F32, tag=f"usb{hi}")
                        nc.vector.tensor_copy(usb, u[hi])
                        usbs.append(usb)
                    rbs = []
                    for hi in range(2):
                        # custom DVE ops mis-read inputs at partition base 64:
                        # copy the denominator row down to partition 0 first
                        d1 = rcp_pool.tile([1, 512], F32, tag=f"d{hi}")
                        nc.vector.tensor_copy(d1, usbs[hi][64:65, :])
                        r1 = rcp_pool.tile([1, 512], F32, tag=f"r{hi}")
                        nc.vector.reciprocal_approx_fast(r1, d1)
                        # gpsimd broadcast of 1/denom across 64 partitions
                        dbc = usb_pool.tile([64, 512], F32, tag=f"dbc{hi}")
                        nc.gpsimd.partition_broadcast(dbc, r1)
                        rbs.append(dbc)
                    for hi in range(2):
                        h = hc * 2 + hi
                        nc.vector.tensor_mul(
                            ot64[0:64, h, qs0:qs0 + 512], usbs[hi][0:64, :], rbs[hi]
                        )
                        # shift-by-one dup for phase D's 128-deep contraction;
                        # slot qs0+511 (g=15) is never read by phase D
                        nc.sync.dma_start(
                            ot64[64:128, h, qs0:qs0 + 511],
                            ot64[0:64, h, qs0 + 1:qs0 + 512],
                        )
                    if hc == 1:
                        # fill PE slack in the scalar-paced hc=1 tiles with
                        # the first head pair's output projection
                        emit_d_chunk(0, 2 * qt)
                        emit_d_chunk(0, 2 * qt + 1)
            for mc in range(NKC):
                emit_d_chunk(1, mc)
            if debug:
                nc.sync.dma_start(ap["ot_dbg"], ot64.bitcast(F32))


def _build(debug=False):
    nc = bacc.Bacc("TRN2", target_bir_lowering=False, debug=False, num_devices=N_CORES)
    ap = {}
    ap["xt"] = nc.dram_tensor("xt", [DM, S], F32, kind="ExternalInput").ap()
    ap["wq"] = nc.dram_tensor("wq", [DM, GD], F32, kind="ExternalInput").ap()
    ap["wk"] = nc.dram_tensor("wk", [DM, GD], F32, kind="ExternalInput").ap()
    ap["wv"] = nc.dram_tensor("wv", [DM, GD], F32, kind="ExternalInput").ap()
    ap["wo"] = nc.dram_tensor("wo", [DM, DM], F32, kind="ExternalInput").ap()
    ap["bq2"] = nc.dram_tensor("bq2", [2, 128], F32, kind="ExternalInput").ap()
    ap["bk2"] = nc.dram_tensor("bk2", [2, 128], F32, kind="ExternalInput").ap()
    ap["bv"] = nc.dram_tensor("bv", [GD], F32, kind="ExternalInput").ap()
    ap["cosb"] = nc.dram_tensor("cosb", [128, S], F32, kind="ExternalInput").ap()
    ap["sinb"] = nc.dram_tensor("sinb", [128, S], F32, kind="ExternalInput").ap()
    ap["rot"] = nc.dram_tensor("rot", [128, 128], F32, kind="ExternalInput").ap()
    ap["ones2"] = nc.dram_tensor("ones2", [2, 128], F32, kind="ExternalInput").ap()
    # per-core output: Y^T [1024, 512] (columns = 4 heads x 128 block rows)
    ap["ypt"] = nc.dram_tensor("ypt", [DM, 512], F32, kind="ExternalOutput").ap()
    if debug:
        ap["qe_dbg"] = nc.dram_tensor("qe_dbg", [2, NST, 128, 512], F32, kind="ExternalOutput").ap()
        ap["ke_dbg"] = nc.dram_tensor("ke_dbg", [2, NST, 128, 512], F32, kind="ExternalOutput").ap()
        ap["v_dbg"] = nc.dram_tensor("v_dbg", [128, NSK, 4, 65], F32, kind="ExternalOutput").ap()
        ap["ot_dbg"] = nc.dram_tensor("ot_dbg", [128, 4, S], F32, kind="ExternalOutput").ap()

    with tile.TileContext(nc) as tc:
        _emit(nc, tc, ap, debug=debug)
    nc.compile()
    return nc


_CACHE = {}


def _rope_tables():
    inv_freq = (1.0 / (10000.0 ** (np.arange(0, HD, 2, dtype=np.float32) / HD))).astype(np.float32)
    t = np.arange(S, dtype=np.float32)
    freqs = np.outer(t, inv_freq).astype(np.float32)  # [S, 32]
    emb = np.concatenate([freqs, freqs], axis=-1)  # [S, 64]
    cosT = np.cos(emb).astype(np.float32).T  # [64, S]
    sinT = np.sin(emb).astype(np.float32).T
    cosb = np.ascontiguousarray(np.concatenate([cosT, cosT], axis=0))  # [128, S]
    sinb = np.ascontiguousarray(np.concatenate([sinT, sinT], axis=0))
    return cosb, sinb


def _rot_matrix():
    p64 = np.zeros((HD, HD), dtype=np.float32)
    for i in range(32):
        p64[i, i + 32] = -1.0
        p64[i + 32, i] = 1.0
    p = np.zeros((128, 128), dtype=np.float32)
    p[0:64, 0:64] = p64
    p[64:128, 64:128] = p64
    return np.ascontiguousarray(p.T)  # lhsT = P^T


def kernel(x, Wq, bq, Wk, bk, Wv, bv, Wo, bo):
    x = np.asarray(x, dtype=np.float32)
    Wq, bq = np.asarray(Wq, np.float32), np.asarray(bq, np.float32)
    Wk, bk = np.asarray(Wk, np.float32), np.asarray(bk, np.float32)
    Wv, bv = np.asarray(Wv, np.float32), np.asarray(bv, np.float32)
    Wo, bo = np.asarray(Wo, np.float32), np.asarray(bo, np.float32)

    if "nc" not in _CACHE:
        _CACHE["nc"] = _build()
    nc = _CACHE["nc"]

    cosb, sinb = _rope_tables()
    rot = _rot_matrix()
    ones2 = np.zeros((2, 128), dtype=np.float32)
    ones2[0, 0:64] = 1.0
    ones2[1, 64:128] = 1.0
    xt_b = [np.ascontiguousarray(x[b].T) for b in range(B)]  # [DM, S]
    wo_c = np.ascontiguousarray(Wo)

    in_maps = []
    for c in range(N_CORES):
        b, hg = divmod(c, HG)
        sl = slice(hg * GD, (hg + 1) * GD)
        in_maps.append(
            {
                "xt": xt_b[b],
                "wq": np.ascontiguousarray(Wq[:, sl]),
                "wk": np.ascontiguousarray(Wk[:, sl]),
                "wv": np.ascontiguousarray(Wv[:, sl]),
                "wo": wo_c,
                "bq2": np.ascontiguousarray(bq[sl].reshape(2, 128)),
                "bk2": np.ascontiguousarray(bk[sl].reshape(2, 128)),
                "bv": np.ascontiguousarray(bv[sl]),
                "cosb": cosb,
                "sinb": sinb,
                "rot": rot,
                "ones2": ones2,
            }
        )

    res = bass_utils.run_bass_kernel_spmd(nc, in_maps, core_ids=list(range(N_CORES)))
    _CACHE["last_results"] = res

    # Block placement: core (b, hg), local head hl -> global head h = hg*4+hl,
    # lands at out[h//8, (h%8)*256 + b*128 : +128, :].
    out = np.empty((B, S, DM), dtype=np.float32)
    for c in range(N_CORES):
        b, hg = divmod(c, HG)
        ypt = res.results[c]["ypt"]  # [1024, 512]
        for hl in range(4):
            h = hg * 4 + hl
            b2 = h // 8
            s2 = (h % 8) * 256 + b * 128
            out[b2, s2:s2 + 128, :] = ypt[:, hl * 128:(hl + 1) * 128].T
    out += bo[None, None, :]
    return out


# revision 45
# speedup vs baseline: 1.0378x; 1.0038x over previous
"""Multi-head attention (RoPE) Trainium2 Bass kernel.

Problem: B=2, S=2048, d_model=1024, 16 heads x head_dim 64, fp32.

The reference faithfully replicates a torch rank-5 reshape bug: the
attention output [1,H,B,S,D] is transposed to (0,2,1,3,4) and
flat-reshaped to [B,S,H*D] BEFORE the Wo projection. Net semantics:
  out[b2, s2, :] = flatten(O[b, h, s0:s0+16, :]) @ Wo + bo
  with h = b2*8 + s2//256, b = (s2//128)%2, s0 = (s2%128)*16,
so the projection is PER-HEAD (contraction mixes 16 seq x 64 dims of one
head) and every (b,h) yields an independent [128, 1024] output block.

Sharding (8 cores): batch (2) x head groups (4 groups of 4 heads).
Per core: QKV slices via f32r matmuls in transposed layout, RoPE
(rotate-half via a signed permutation matmul), per-head attention with
unnormalized softmax (ones-column appended to V gives the denominator),
normalize into ot64 [64, 4head, S], then per-head scrambled projection
against full Wo. Host places the 32 independent blocks and adds bo.

Scheduling notes (v2):
 - weight/x DMAs are chunked and spread over 4 queues so the first
   matmul starts ~4us in instead of ~20us.
 - softmax normalization: reciprocal_approx_fast on the denominator row
   + a tiny K=2 matmul that broadcasts both heads' 1/denom rows across
   64 partitions (replaces a DRAM round-trip partition_broadcast and a
   16x slower vector.reciprocal).
 - phase D contracts 128-deep: Wo rows for seq pair (2t, 2t+1) live in
   partitions 0-63 / 64-127; the duplicate copy of O^T in partitions
   64-127 is written shifted by one seq position so a single rhs AP
   covers both contraction halves.
 - phase D is split by head pair: D(hc=0) is emitted interleaved into
   phase C's hc=1 tiles (the PE has slack there; phase C is paced by the
   scalar engine's exp), leaving only D(hc=1) as the serial tail.
"""

import numpy as np

import concourse.bass as bass
import concourse.tile as tile
from concourse import bacc, mybir
from concourse import bass_utils

F32 = mybir.dt.float32
MM_DT = mybir.dt.float32r  # matmul operand dtype (float32r: 1 cyc/row)

B, S, DM, H, HD = 2, 2048, 1024, 16, 64
N_CORES = 8
HG = 4          # head groups (tensor-parallel factor)
GD = DM // HG   # qkv dims per core = 256
NKC = DM // 128   # d_model contraction chunks = 8
NST = S // 512    # seq tiles of 512 = 4
NSK = S // 128    # seq_k chunks of 128 = 16


def _emit(nc, tc, ap, debug=False):
    import contextlib

    ctx = contextlib.ExitStack()
    with ctx:
        consts = ctx.enter_context(tc.tile_pool(name="consts", bufs=1))
        bigp = ctx.enter_context(tc.tile_pool(name="big", bufs=1))

        # ---- persistent tiles ----
        # ones2[hi, m] = 1 where m//64 == hi: K=2 matmul broadcasts the two
        # 1/denom rows across partition halves.
        ones2 = consts.tile([2, 128], MM_DT)
        nc.gpsimd.dma_start(ones2, ap["ones2"].bitcast(MM_DT))

        # qe/ke split per (mc=head-pair, st) for fine-grained deps
        qe_t = [
            [bigp.tile([128, 512], MM_DT, name=f"qe{mc}_{st}") for st in range(NST)]
            for mc in range(2)
        ]
        ke_t = [
            [bigp.tile([128, 512], MM_DT, name=f"ke{mc}_{st}") for st in range(NST)]
            for mc in range(2)
        ]
        # Zero-padded ke per head: 64-partition-contraction f32r matmuls
        # stream at HALF rate on TRN2, so scores use full-128 contraction with
        # the other head's partitions zeroed. ke_z[hc][st][:, hi, :] holds
        # head hi's rows live, the other 64 partitions zero.
        ke_z = [
            [bigp.tile([128, 2, 512], MM_DT, name=f"kez{hc}_{st}") for st in range(NST)]
            for hc in range(2)
        ]
        for hc in range(2):
            for st in range(NST):
                nc.vector.memset(ke_z[hc][st][64:128, 0, :].bitcast(F32), 0.0)
                nc.vector.memset(ke_z[hc][st][0:64, 1, :].bitcast(F32), 0.0)
        # V natural layout + ones column: [128 seq, kc, head, 65]
        vsb = bigp.tile([128, NSK, 4, 65], MM_DT, name="vsb", tag="vsb")
        nc.vector.memset(vsb[:, :, :, 64:65].bitcast(F32), 1.0)
        # normalized attention output, heads on the free axis: [128, head, S].
        # Partitions 0-63 hold O^T; 64-127 hold a copy SHIFTED BY ONE seq
        # position (dup[64+d, h, s] = O^T[d, h, s+1]) so phase D can contract
        # seq pairs (2t, 2t+1) 128-deep with a single rhs AP.
        ot64 = bigp.tile([128, 4, S], MM_DT, name="ot64", tag="ot64")

        with tc.tile_pool(name="bconsts", bufs=1) as bconsts:
            # ---- weights to SBUF, chunked + spread across queues ----
            wq = bconsts.tile([128, NKC, GD], MM_DT)
            wqr = ap["wq"].rearrange("(kc p) m -> p kc m", p=128).bitcast(MM_DT)
            wk = bconsts.tile([128, NKC, GD], MM_DT)
            wkr = ap["wk"].rearrange("(kc p) m -> p kc m", p=128).bitcast(MM_DT)
            wv = bconsts.tile([128, NKC, GD], MM_DT)
            wvr = ap["wv"].rearrange("(kc p) m -> p kc m", p=128).bitcast(MM_DT)
            rot = bconsts.tile([128, 128], MM_DT)
            nc.gpsimd.dma_start(rot, ap["rot"].bitcast(MM_DT))
            bqc = bconsts.tile([128, 2], F32)
            nc.gpsimd.dma_start(bqc, ap["bq2"].rearrange("c p -> p c"))
            bkc = bconsts.tile([128, 2], F32)
            nc.gpsimd.dma_start(bkc, ap["bk2"].rearrange("c p -> p c"))
            bvb = bconsts.tile([128, GD], F32)
            nc.gpsimd.dma_start(bvb, ap["bv"].partition_broadcast(128))
            # critical-first: small leading chunks of wq/wk so kc0's matmuls
            # start ASAP; the rest as few big DMAs (each dma_start costs
            # ~650ns of queue issue time)
            nc.sync.dma_start(wq[:, 0:2, :], wqr[:, 0:2, :])
            nc.sync.dma_start(wk[:, 0:2, :], wkr[:, 0:2, :])
            nc.sync.dma_start(wq[:, 2:8, :], wqr[:, 2:8, :])
            nc.sync.dma_start(wk[:, 2:8, :], wkr[:, 2:8, :])
            # RoPE tables: DMAs deferred until after st0's matmul emission
            cosb = bconsts.tile([128, S], F32)
            sinb = bconsts.tile([128, S], F32)

            # ================= Phase B: QKV projections + RoPE =============
            xtr = ap["xt"].rearrange("(k2 p2 p) s -> p k2 p2 s", p=128, p2=2)
            with (
                tc.tile_pool(name="xt", bufs=6) as xt_pool,
                tc.tile_pool(name="raw", bufs=1) as raw_pool,
                tc.tile_pool(name="t1", bufs=2) as t1_pool,
                tc.tile_pool(name="ps_qk", bufs=1, space="PSUM") as ps_qk,
                tc.tile_pool(name="ps_v", bufs=1, space="PSUM") as ps_v,
            ):
                for st in range(NST):
                    sl = slice(st * 512, (st + 1) * 512)
                    pqk = {}
                    pv = {}
                    for tgt in range(2):
                        for mc in range(2):
                            pqk[tgt, mc] = ps_qk.tile(
                                [128, 512], F32, name=f"pqk{tgt}{mc}", tag=f"qk{tgt}{mc}"
                            )
                    for ss in range(4):
                        pv[ss] = ps_v.tile([128, GD], F32, name=f"pv{ss}", tag=f"v{ss}")
                    xts = []
                    for kc2 in range(NKC // 2):
                        xt2 = xt_pool.tile([128, 2, 512], MM_DT)
                        xts.append(xt2)
                        if st == 0:
                            q = [nc.scalar, nc.gpsimd][kc2 % 2]
                        else:
                            q = [nc.scalar, nc.gpsimd, nc.sync][(st * 4 + kc2) % 3]
                        q.dma_start(xt2, xtr[:, kc2, :, sl].bitcast(MM_DT))
                        if st == 0 and kc2 == 0:
                            # wv needed only after the whole QK block
                            nc.scalar.dma_start(wv[:, 0:2, :], wvr[:, 0:2, :])
                            nc.sync.dma_start(wv[:, 2:8, :], wvr[:, 2:8, :])
                        for j in range(2):
                            kc = kc2 * 2 + j
                            for tgt in range(2):
                                w_sb = wq if tgt == 0 else wk
                                for mc in range(2):
                                    nc.tensor.matmul(
                                        pqk[tgt, mc],
                                        lhsT=w_sb[:, kc, mc * 128:(mc + 1) * 128],
                                        rhs=xt2[:, j, :],
                                        start=(kc == 0),
                                        stop=(kc == NKC - 1),
                                    )
                    # V after the full QK block: decouples from the wv DMA
                    # and from pv-bank availability at tile start
                    for kc2 in range(NKC // 2):
                        for j in range(2):
                            kc = kc2 * 2 + j
                            for ss in range(4):
                                nc.tensor.matmul(
                                    pv[ss],
                                    lhsT=xts[kc2][:, j, ss * 128:(ss + 1) * 128],
                                    rhs=wv[:, kc, :],
                                    start=(kc == 0),
                                    stop=(kc == NKC - 1),
                                )
                    if st == 0:
                        # RoPE tables: needed at first rope writeback (~21us)
                        nc.sync.dma_start(cosb, ap["cosb"])
                        nc.gpsimd.dma_start(sinb, ap["sinb"])
                    # issue all accumulator drains first: the QK psum slots gate
                    # the next seq tile's matmuls, so don't interleave the slower
                    # RoPE chain between them
                    raws = {}
                    for tgt in range(2):
                        bias = bqc if tgt == 0 else bkc
                        for mc in range(2):
                            raw = raw_pool.tile(
                                [128, 512], MM_DT, name=f"raw{tgt}{mc}", tag=f"raw{tgt}{mc}"
                            )
                            nc.vector.tensor_scalar_add(raw, pqk[tgt, mc], bias[:, mc:mc + 1])
                            raws[tgt, mc] = raw
                    for ss in range(4):
                        nc.vector.tensor_add(
                            vsb[:, st * 4 + ss, :, 0:64],
                            pv[ss].rearrange("p (h d) -> p h d", h=4),
                            bvb.rearrange("p (h d) -> p h d", h=4),
                        )
                    rpss = {}
                    for tgt in (1, 0):  # K first: phase C needs ke_z earliest
                        for mc in range(2):
                            rps = ps_v.tile(
                                [128, 512], F32, name=f"rps{tgt}{mc}", tag=f"v{tgt * 2 + mc}"
                            )
                            nc.tensor.matmul(rps, lhsT=rot, rhs=raws[tgt, mc], start=True, stop=True)
                            rpss[tgt, mc] = rps
                    for tgt in (1, 0):
                        dst = qe_t if tgt == 0 else ke_t
                        for mc in range(2):
                            t1 = t1_pool.tile([128, 512], F32)
                            nc.vector.tensor_mul(t1, rpss[tgt, mc], sinb[:, sl])
                            d = dst[mc][st]
                            nc.vector.tensor_mul(d, raws[tgt, mc], cosb[:, sl])
                            nc.vector.tensor_add(d, d, t1)
                            if tgt == 1:
                                # scatter ke halves into the zero-padded tiles
                                qz = nc.gpsimd if mc == 0 else nc.sync
                                qz.dma_start(ke_z[mc][st][0:64, 0, :], d[0:64, :])
                                qz.dma_start(ke_z[mc][st][64:128, 1, :], d[64:128, :])

        if debug:
            for mc in range(2):
                for st in range(NST):
                    nc.sync.dma_start(ap["qe_dbg"][mc, st], qe_t[mc][st].bitcast(F32))
                    nc.sync.dma_start(ap["ke_dbg"][mc, st], ke_t[mc][st].bitcast(F32))
            nc.sync.dma_start(ap["v_dbg"], vsb.bitcast(F32))

        # ================= Phase C: attention (+ interleaved phase D) =====
        # phase D view: ot_rp[p, h, s2r, g] = ot64[p, h, s2r*16+g]; the rhs for
        # contraction pair t reads g=2t across all 128 partitions (upper half
        # holds the shift-by-one dup = seq 2t+1 data).
        ot_rp = ot64.rearrange("p h (s2r g) -> p h s2r g", g=16)
        LAG = 2  # AV matmuls trail score matmuls by LAG kc iterations
        with (
            tc.tile_pool(name="e", bufs=LAG + 2) as e_pool,
            tc.tile_pool(name="usb", bufs=2) as usb_pool,
            tc.tile_pool(name="rcp", bufs=2) as rcp_pool,
            tc.tile_pool(name="wop", bufs=1) as wo_pool,
            tc.tile_pool(name="ysb", bufs=1) as y_pool,
            tc.tile_pool(name="ps_s", bufs=2, space="PSUM") as ps_s,
            tc.tile_pool(name="ps_u", bufs=1, space="PSUM") as ps_u,
            tc.tile_pool(name="ps_y", bufs=1, space="PSUM") as ps_y,
        ):
            # full Wo resident: [128, c(pair), mc, m]; lhsT for (mc, t) is
            # wo_t[:, t, mc, :] = Wo rows 128t..128t+127, cols mc*128..+128
            wo_t = wo_pool.tile([128, 8, NKC, 128], MM_DT, name="wo_t", tag="wo_t")
            for mc in range(NKC):
                nc.sync.dma_start(
                    wo_t[:, :, mc, :],
                    ap["wo"][:, mc * 128:(mc + 1) * 128]
                    .rearrange("(c p) m -> p c m", p=128)
                    .bitcast(MM_DT),
                )

            def emit_d_chunk(hc, mc):
                py = ps_y.tile([128, 256], F32, name=f"py{hc}{mc}", tag=f"py{mc % 2}")
                for t in range(8):
                    nc.tensor.matmul(
                        py,
                        lhsT=wo_t[:, t, mc, :],
                        rhs=ot_rp[:, hc * 2:hc * 2 + 2, :, 2 * t],
                        start=(t == 0),
                        stop=(t == 7),
                    )
                ysb = y_pool.tile([128, 256], F32, name="ysb", tag=f"ysb{mc % 2}")
                nc.vector.tensor_copy(ysb, py)
                nc.sync.dma_start(
                    ap["ypt"][mc * 128:(mc + 1) * 128, hc * 256:(hc + 1) * 256], ysb
                )

            for hc in range(2):
                for qt in range(NST):
                    qs0 = qt * 512
                    u = [
                        ps_u.tile([65, 512], F32, name=f"u{hi}", tag=f"u{hi}")
                        for hi in range(2)
                    ]
                    es = {}
                    for kc in range(NSK + LAG):
                        if kc >= LAG:
                            ka = kc - LAG
                            for hi in range(2):
                                nc.tensor.matmul(
                                    u[hi],
                                    lhsT=vsb[:, ka, hc * 2 + hi, :],
                                    rhs=es[ka][:, hi * 512:(hi + 1) * 512],
                                    start=(ka == 0),
                                    stop=(ka == NSK - 1),
                                )
                            if ka > 0:
                                del es[ka - 1]
                        if kc < NSK:
                            # both heads' scores side by side in one 2-bank group;
                            # full-128 contraction via zero-padded ke (64-deep
                            # f32r matmuls stream at half rate)
                            g = ps_s.tile([128, 1024], F32, tag="sg", name="sg")
                            for hi in range(2):
                                nc.tensor.matmul(
                                    g[:, hi * 512:(hi + 1) * 512],
                                    lhsT=ke_z[hc][kc // 4][:, hi, (kc % 4) * 128:(kc % 4 + 1) * 128],
                                    rhs=qe_t[hc][qt],
                                    start=True,
                                    stop=True,
                                )
                            e = e_pool.tile([128, 1024], MM_DT, name="e", tag="e")
                            nc.scalar.activation(
                                e, g, mybir.ActivationFunctionType.Exp, scale=0.125
                            )
                            es[kc] = e
                    # ---- normalize: ot = U[0:64] * (1/U[64]) ----
                    # denominator rows first so recip/broadcast overlap the
                    # bulk usb copies (custom DVE ops mis-read partition-base-
                    # 64 inputs, so stage them at partition 0)
                    d1s = []
                    for hi in range(2):
                        d1 = rcp_pool.tile([1, 512], F32, tag=f"d{hi}")
                        nc.vector.tensor_copy(d1, u[hi][64:65, :])
                        d1s.append(d1)
                    rbs = []
                    for hi in range(2):
                        r1 = rcp_pool.tile([1, 512], F32, tag=f"r{hi}")
                        nc.vector.reciprocal_approx_fast(r1, d1s[hi])
                        # gpsimd broadcast of 1/denom across 64 partitions
                        dbc = usb_pool.tile([64, 512], F32, tag=f"dbc{hi}")
                        nc.gpsimd.partition_broadcast(dbc, r1)
                        rbs.append(dbc)
                    usbs = []
                    for hi in range(2):
                        usb = usb_pool.tile([64, 512], F32, tag=f"usb{hi}")
                        nc.vector.tensor_copy(usb, u[hi][0:64, :])
                        usbs.append(usb)
                    for hi in range(2):
                        h = hc * 2 + hi
                        nc.vector.tensor_mul(
                            ot64[0:64, h, qs0:qs0 + 512], usbs[hi], rbs[hi]
                        )
                        # shift-by-one dup for phase D's 128-deep contraction;
                        # slot qs0+511 (g=15) is never read by phase D
                        nc.sync.dma_start(
                            ot64[64:128, h, qs0:qs0 + 511],
                            ot64[0:64, h, qs0 + 1:qs0 + 512],
                        )
                    if hc == 1:
                        # fill PE slack in the scalar-paced hc=1 tiles with
                        # the first head pair's output projection
                        emit_d_chunk(0, 2 * qt)
                        emit_d_chunk(0, 2 * qt + 1)
            for mc in range(NKC):
                emit_d_chunk(1, mc)
            if debug:
                nc.sync.dma_start(ap["ot_dbg"], ot64.bitcast(F32))


def _build(debug=False):
    nc = bacc.Bacc("TRN2", target_bir_lowering=False, debug=False, num_devices=N_CORES)
    ap = {}
    ap["xt"] = nc.dram_tensor("xt", [DM, S], F32, kind="ExternalInput").ap()
    ap["wq"] = nc.dram_tensor("wq", [DM, GD], F32, kind="ExternalInput").ap()
    ap["wk"] = nc.dram_tensor("wk", [DM, GD], F32, kind="ExternalInput").ap()
    ap["wv"] = nc.dram_tensor("wv", [DM, GD], F32, kind="ExternalInput").ap()
    ap["wo"] = nc.dram_tensor("wo", [DM, DM], F32, kind="ExternalInput").ap()
    ap["bq2"] = nc.dram_tensor("bq2", [2, 128], F32, kind="ExternalInput").ap()
    ap["bk2"] = nc.dram_tensor("bk2", [2, 128], F32, kind="ExternalInput").ap()
    ap["bv"] = nc.dram_tensor("bv", [GD], F32, kind="ExternalInput").ap()
    ap["cosb"] = nc.dram_tensor("cosb", [128, S], F32, kind="ExternalInput").ap()
    ap["sinb"] = nc.dram_tensor("sinb", [128, S], F32, kind="ExternalInput").ap()
    ap["rot"] = nc.dram_tensor("rot", [128, 128], F32, kind="ExternalInput").ap()
    ap["ones2"] = nc.dram_tensor("ones2", [2, 128], F32, kind="ExternalInput").ap()
    # per-core output: Y^T [1024, 512] (columns = 4 heads x 128 block rows)
    ap["ypt"] = nc.dram_tensor("ypt", [DM, 512], F32, kind="ExternalOutput").ap()
    if debug:
        ap["qe_dbg"] = nc.dram_tensor("qe_dbg", [2, NST, 128, 512], F32, kind="ExternalOutput").ap()
        ap["ke_dbg"] = nc.dram_tensor("ke_dbg", [2, NST, 128, 512], F32, kind="ExternalOutput").ap()
        ap["v_dbg"] = nc.dram_tensor("v_dbg", [128, NSK, 4, 65], F32, kind="ExternalOutput").ap()
        ap["ot_dbg"] = nc.dram_tensor("ot_dbg", [128, 4, S], F32, kind="ExternalOutput").ap()

    with tile.TileContext(nc) as tc:
        _emit(nc, tc, ap, debug=debug)
    nc.compile()
    return nc


_CACHE = {}


def _rope_tables():
    inv_freq = (1.0 / (10000.0 ** (np.arange(0, HD, 2, dtype=np.float32) / HD))).astype(np.float32)
    t = np.arange(S, dtype=np.float32)
    freqs = np.outer(t, inv_freq).astype(np.float32)  # [S, 32]
    emb = np.concatenate([freqs, freqs], axis=-1)  # [S, 64]
    cosT = np.cos(emb).astype(np.float32).T  # [64, S]
    sinT = np.sin(emb).astype(np.float32).T
    cosb = np.ascontiguousarray(np.concatenate([cosT, cosT], axis=0))  # [128, S]
    sinb = np.ascontiguousarray(np.concatenate([sinT, sinT], axis=0))
    return cosb, sinb


def _rot_matrix():
    p64 = np.zeros((HD, HD), dtype=np.float32)
    for i in range(32):
        p64[i, i + 32] = -1.0
        p64[i + 32, i] = 1.0
    p = np.zeros((128, 128), dtype=np.float32)
    p[0:64, 0:64] = p64
    p[64:128, 64:128] = p64
    return np.ascontiguousarray(p.T)  # lhsT = P^T


def kernel(x, Wq, bq, Wk, bk, Wv, bv, Wo, bo):
    x = np.asarray(x, dtype=np.float32)
    Wq, bq = np.asarray(Wq, np.float32), np.asarray(bq, np.float32)
    Wk, bk = np.asarray(Wk, np.float32), np.asarray(bk, np.float32)
    Wv, bv = np.asarray(Wv, np.float32), np.asarray(bv, np.float32)
    Wo, bo = np.asarray(Wo, np.float32), np.asarray(bo, np.float32)

    if "nc" not in _CACHE:
        _CACHE["nc"] = _build()
    nc = _CACHE["nc"]

    cosb, sinb = _rope_tables()
    rot = _rot_matrix()
    ones2 = np.zeros((2, 128), dtype=np.float32)
    ones2[0, 0:64] = 1.0
    ones2[1, 64:128] = 1.0
    xt_b = [np.ascontiguousarray(x[b].T) for b in range(B)]  # [DM, S]
    wo_c = np.ascontiguousarray(Wo)

    in_maps = []
    for c in range(N_CORES):
        b, hg = divmod(c, HG)
        sl = slice(hg * GD, (hg + 1) * GD)
        in_maps.append(
            {
                "xt": xt_b[b],
                "wq": np.ascontiguousarray(Wq[:, sl]),
                "wk": np.ascontiguousarray(Wk[:, sl]),
                "wv": np.ascontiguousarray(Wv[:, sl]),
                "wo": wo_c,
                "bq2": np.ascontiguousarray(bq[sl].reshape(2, 128)),
                "bk2": np.ascontiguousarray(bk[sl].reshape(2, 128)),
                "bv": np.ascontiguousarray(bv[sl]),
                "cosb": cosb,
                "sinb": sinb,
                "rot": rot,
                "ones2": ones2,
            }
        )

    res = bass_utils.run_bass_kernel_spmd(nc, in_maps, core_ids=list(range(N_CORES)))
    _CACHE["last_results"] = res

    # Block placement: core (b, hg), local head hl -> global head h = hg*4+hl,
    # lands at out[h//8, (h%8)*256 + b*128 : +128, :].
    out = np.empty((B, S, DM), dtype=np.float32)
    for c in range(N_CORES):
        b, hg = divmod(c, HG)
        ypt = res.results[c]["ypt"]  # [1024, 512]
        for hl in range(4):
            h = hg * 4 + hl
            b2 = h // 8
            s2 = (h % 8) * 256 + b * 128
            out[b2, s2:s2 + 128, :] = ypt[:, hl * 128:(hl + 1) * 128].T
    out += bo[None, None, :]
    return out


# revision 46
# speedup vs baseline: 1.0513x; 1.0130x over previous
"""Multi-head attention (RoPE) Trainium2 Bass kernel.

Problem: B=2, S=2048, d_model=1024, 16 heads x head_dim 64, fp32.

The reference faithfully replicates a torch rank-5 reshape bug: the
attention output [1,H,B,S,D] is transposed to (0,2,1,3,4) and
flat-reshaped to [B,S,H*D] BEFORE the Wo projection. Net semantics:
  out[b2, s2, :] = flatten(O[b, h, s0:s0+16, :]) @ Wo + bo
  with h = b2*8 + s2//256, b = (s2//128)%2, s0 = (s2%128)*16,
so the projection is PER-HEAD (contraction mixes 16 seq x 64 dims of one
head) and every (b,h) yields an independent [128, 1024] output block.

Sharding (8 cores): batch (2) x head groups (4 groups of 4 heads).
Per core: QKV slices via f32r matmuls in transposed layout, RoPE
(rotate-half via a signed permutation matmul), per-head attention with
unnormalized softmax (ones-column appended to V gives the denominator),
normalize into ot64 [64, 4head, S], then per-head scrambled projection
against full Wo. Host places the 32 independent blocks and adds bo.

Scheduling notes (v2):
 - weight/x DMAs are chunked and spread over 4 queues so the first
   matmul starts ~4us in instead of ~20us.
 - softmax normalization: reciprocal_approx_fast on the denominator row
   + a tiny K=2 matmul that broadcasts both heads' 1/denom rows across
   64 partitions (replaces a DRAM round-trip partition_broadcast and a
   16x slower vector.reciprocal).
 - phase D contracts 128-deep: Wo rows for seq pair (2t, 2t+1) live in
   partitions 0-63 / 64-127; the duplicate copy of O^T in partitions
   64-127 is written shifted by one seq position so a single rhs AP
   covers both contraction halves.
 - phase D is split by head pair: D(hc=0) is emitted interleaved into
   phase C's hc=1 tiles (the PE has slack there; phase C is paced by the
   scalar engine's exp), leaving only D(hc=1) as the serial tail.
"""

import numpy as np

import concourse.bass as bass
import concourse.tile as tile
from concourse import bacc, mybir
from concourse import bass_utils

F32 = mybir.dt.float32
MM_DT = mybir.dt.float32r  # matmul operand dtype (float32r: 1 cyc/row)

B, S, DM, H, HD = 2, 2048, 1024, 16, 64
N_CORES = 8
HG = 4          # head groups (tensor-parallel factor)
GD = DM // HG   # qkv dims per core = 256
NKC = DM // 128   # d_model contraction chunks = 8
NST = S // 512    # seq tiles of 512 = 4
NSK = S // 128    # seq_k chunks of 128 = 16


def _emit(nc, tc, ap, debug=False):
    import contextlib

    ctx = contextlib.ExitStack()
    with ctx:
        consts = ctx.enter_context(tc.tile_pool(name="consts", bufs=1))
        bigp = ctx.enter_context(tc.tile_pool(name="big", bufs=1))

        # ---- persistent tiles ----
        # ones2[hi, m] = 1 where m//64 == hi: K=2 matmul broadcasts the two
        # 1/denom rows across partition halves.
        ones2 = consts.tile([2, 128], MM_DT)
        nc.gpsimd.dma_start(ones2, ap["ones2"].bitcast(MM_DT))

        # qe/ke split per (mc=head-pair, st) for fine-grained deps
        qe_t = [
            [bigp.tile([128, 512], MM_DT, name=f"qe{mc}_{st}") for st in range(NST)]
            for mc in range(2)
        ]
        ke_t = [
            [bigp.tile([128, 512], MM_DT, name=f"ke{mc}_{st}") for st in range(NST)]
            for mc in range(2)
        ]
        # Zero-padded ke per head: 64-partition-contraction f32r matmuls
        # stream at HALF rate on TRN2, so scores use full-128 contraction with
        # the other head's partitions zeroed. ke_z[hc][st][:, hi, :] holds
        # head hi's rows live, the other 64 partitions zero.
        ke_z = [
            [bigp.tile([128, 2, 512], MM_DT, name=f"kez{hc}_{st}") for st in range(NST)]
            for hc in range(2)
        ]
        for hc in range(2):
            for st in range(NST):
                nc.vector.memset(ke_z[hc][st][64:128, 0, :].bitcast(F32), 0.0)
                nc.vector.memset(ke_z[hc][st][0:64, 1, :].bitcast(F32), 0.0)
        # V natural layout + ones column: [128 seq, kc, head, 65]
        vsb = bigp.tile([128, NSK, 4, 65], MM_DT, name="vsb", tag="vsb")
        nc.vector.memset(vsb[:, :, :, 64:65].bitcast(F32), 1.0)
        # normalized attention output, heads on the free axis: [128, head, S].
        # Partitions 0-63 hold O^T; 64-127 hold a copy SHIFTED BY ONE seq
        # position (dup[64+d, h, s] = O^T[d, h, s+1]) so phase D can contract
        # seq pairs (2t, 2t+1) 128-deep with a single rhs AP.
        ot64 = bigp.tile([128, 4, S], MM_DT, name="ot64", tag="ot64")

        with tc.tile_pool(name="bconsts", bufs=1) as bconsts:
            # ---- weights to SBUF, chunked + spread across queues ----
            wq = bconsts.tile([128, NKC, GD], MM_DT)
            wqr = ap["wq"].rearrange("(kc p) m -> p kc m", p=128).bitcast(MM_DT)
            wk = bconsts.tile([128, NKC, GD], MM_DT)
            wkr = ap["wk"].rearrange("(kc p) m -> p kc m", p=128).bitcast(MM_DT)
            wv = bconsts.tile([128, NKC, GD], MM_DT)
            wvr = ap["wv"].rearrange("(kc p) m -> p kc m", p=128).bitcast(MM_DT)
            rot = bconsts.tile([128, 128], MM_DT)
            nc.gpsimd.dma_start(rot, ap["rot"].bitcast(MM_DT))
            bqc = bconsts.tile([128, 2], F32)
            nc.gpsimd.dma_start(bqc, ap["bq2"].rearrange("c p -> p c"))
            bkc = bconsts.tile([128, 2], F32)
            nc.gpsimd.dma_start(bkc, ap["bk2"].rearrange("c p -> p c"))
            bvb = bconsts.tile([128, GD], F32)
            nc.gpsimd.dma_start(bvb, ap["bv"].partition_broadcast(128))
            # critical-first: small leading chunks of wq/wk so kc0's matmuls
            # start ASAP; the rest as few big DMAs (each dma_start costs
            # ~650ns of queue issue time)
            nc.sync.dma_start(wq[:, 0:2, :], wqr[:, 0:2, :])
            nc.sync.dma_start(wk[:, 0:2, :], wkr[:, 0:2, :])
            nc.sync.dma_start(wq[:, 2:8, :], wqr[:, 2:8, :])
            nc.sync.dma_start(wk[:, 2:8, :], wkr[:, 2:8, :])
            # RoPE tables: DMAs deferred until after st0's matmul emission
            cosb = bconsts.tile([128, S], F32)
            sinb = bconsts.tile([128, S], F32)

            # ================= Phase B: QKV projections + RoPE =============
            xtr = ap["xt"].rearrange("(k2 p2 p) s -> p k2 p2 s", p=128, p2=2)
            with (
                tc.tile_pool(name="xt", bufs=6) as xt_pool,
                tc.tile_pool(name="raw", bufs=1) as raw_pool,
                tc.tile_pool(name="t1", bufs=2) as t1_pool,
                tc.tile_pool(name="ps_qk", bufs=1, space="PSUM") as ps_qk,
                tc.tile_pool(name="ps_v", bufs=1, space="PSUM") as ps_v,
            ):
                for st in range(NST):
                    sl = slice(st * 512, (st + 1) * 512)
                    pqk = {}
                    pv = {}
                    for tgt in range(2):
                        for mc in range(2):
                            pqk[tgt, mc] = ps_qk.tile(
                                [128, 512], F32, name=f"pqk{tgt}{mc}", tag=f"qk{tgt}{mc}"
                            )
                    for ss in range(4):
                        pv[ss] = ps_v.tile([128, GD], F32, name=f"pv{ss}", tag=f"v{ss}")
                    xts = []
                    for kc2 in range(NKC // 2):
                        xt2 = xt_pool.tile([128, 2, 512], MM_DT)
                        xts.append(xt2)
                        if st == 0:
                            q = [nc.scalar, nc.gpsimd][kc2 % 2]
                        else:
                            q = [nc.scalar, nc.gpsimd, nc.sync][(st * 4 + kc2) % 3]
                        q.dma_start(xt2, xtr[:, kc2, :, sl].bitcast(MM_DT))
                        if st == 0 and kc2 == 0:
                            # wv needed only after the whole QK block
                            nc.scalar.dma_start(wv[:, 0:2, :], wvr[:, 0:2, :])
                            nc.sync.dma_start(wv[:, 2:8, :], wvr[:, 2:8, :])
                        for j in range(2):
                            kc = kc2 * 2 + j
                            for tgt in range(2):
                                w_sb = wq if tgt == 0 else wk
                                for mc in range(2):
                                    nc.tensor.matmul(
                                        pqk[tgt, mc],
                                        lhsT=w_sb[:, kc, mc * 128:(mc + 1) * 128],
                                        rhs=xt2[:, j, :],
                                        start=(kc == 0),
                                        stop=(kc == NKC - 1),
                                    )
                    # V after the full QK block: decouples from the wv DMA
                    # and from pv-bank availability at tile start
                    for kc2 in range(NKC // 2):
                        for j in range(2):
                            kc = kc2 * 2 + j
                            for ss in range(4):
                                nc.tensor.matmul(
                                    pv[ss],
                                    lhsT=xts[kc2][:, j, ss * 128:(ss + 1) * 128],
                                    rhs=wv[:, kc, :],
                                    start=(kc == 0),
                                    stop=(kc == NKC - 1),
                                )
                    if st == 0:
                        # RoPE tables: needed at first rope writeback (~21us).
                        # Both on sync: a 1MB issue-stall on gpsimd/scalar
                        # would block the st1 xt prefetch behind it.
                        nc.sync.dma_start(cosb, ap["cosb"])
                        nc.sync.dma_start(sinb, ap["sinb"])
                    # issue all accumulator drains first: the QK psum slots gate
                    # the next seq tile's matmuls, so don't interleave the slower
                    # RoPE chain between them
                    raws = {}
                    for tgt in range(2):
                        bias = bqc if tgt == 0 else bkc
                        for mc in range(2):
                            raw = raw_pool.tile(
                                [128, 512], MM_DT, name=f"raw{tgt}{mc}", tag=f"raw{tgt}{mc}"
                            )
                            nc.vector.tensor_scalar_add(raw, pqk[tgt, mc], bias[:, mc:mc + 1])
                            raws[tgt, mc] = raw
                    for ss in range(4):
                        nc.vector.tensor_add(
                            vsb[:, st * 4 + ss, :, 0:64],
                            pv[ss].rearrange("p (h d) -> p h d", h=4),
                            bvb.rearrange("p (h d) -> p h d", h=4),
                        )
                    rpss = {}
                    for tgt in (1, 0):  # K first: phase C needs ke_z earliest
                        for mc in range(2):
                            rps = ps_v.tile(
                                [128, 512], F32, name=f"rps{tgt}{mc}", tag=f"v{tgt * 2 + mc}"
                            )
                            nc.tensor.matmul(rps, lhsT=rot, rhs=raws[tgt, mc], start=True, stop=True)
                            rpss[tgt, mc] = rps
                    for tgt in (1, 0):
                        dst = qe_t if tgt == 0 else ke_t
                        for mc in range(2):
                            t1 = t1_pool.tile([128, 512], F32)
                            nc.vector.tensor_mul(t1, rpss[tgt, mc], sinb[:, sl])
                            d = dst[mc][st]
                            nc.vector.tensor_mul(d, raws[tgt, mc], cosb[:, sl])
                            nc.vector.tensor_add(d, d, t1)
                            if tgt == 1:
                                # scatter ke halves into the zero-padded tiles
                                qz = nc.gpsimd if mc == 0 else nc.sync
                                qz.dma_start(ke_z[mc][st][0:64, 0, :], d[0:64, :])
                                qz.dma_start(ke_z[mc][st][64:128, 1, :], d[64:128, :])

        if debug:
            for mc in range(2):
                for st in range(NST):
                    nc.sync.dma_start(ap["qe_dbg"][mc, st], qe_t[mc][st].bitcast(F32))
                    nc.sync.dma_start(ap["ke_dbg"][mc, st], ke_t[mc][st].bitcast(F32))
            nc.sync.dma_start(ap["v_dbg"], vsb.bitcast(F32))

        # ================= Phase C: attention (+ interleaved phase D) =====
        # phase D view: ot_rp[p, h, s2r, g] = ot64[p, h, s2r*16+g]; the rhs for
        # contraction pair t reads g=2t across all 128 partitions (upper half
        # holds the shift-by-one dup = seq 2t+1 data).
        ot_rp = ot64.rearrange("p h (s2r g) -> p h s2r g", g=16)
        LAG = 2  # AV matmuls trail score matmuls by LAG kc iterations
        with (
            tc.tile_pool(name="e", bufs=LAG + 2) as e_pool,
            tc.tile_pool(name="usb", bufs=2) as usb_pool,
            tc.tile_pool(name="rcp", bufs=2) as rcp_pool,
            tc.tile_pool(name="wop", bufs=1) as wo_pool,
            tc.tile_pool(name="ysb", bufs=1) as y_pool,
            tc.tile_pool(name="ps_s", bufs=2, space="PSUM") as ps_s,
            tc.tile_pool(name="ps_u", bufs=1, space="PSUM") as ps_u,
            tc.tile_pool(name="ps_y", bufs=1, space="PSUM") as ps_y,
        ):
            # full Wo resident: [128, c(pair), mc, m]; lhsT for (mc, t) is
            # wo_t[:, t, mc, :] = Wo rows 128t..128t+127, cols mc*128..+128
            wo_t = wo_pool.tile([128, 8, NKC, 128], MM_DT, name="wo_t", tag="wo_t")
            for mc in range(NKC):
                nc.sync.dma_start(
                    wo_t[:, :, mc, :],
                    ap["wo"][:, mc * 128:(mc + 1) * 128]
                    .rearrange("(c p) m -> p c m", p=128)
                    .bitcast(MM_DT),
                )

            def emit_d_chunk(hc, mc):
                py = ps_y.tile([128, 256], F32, name=f"py{hc}{mc}", tag=f"py{mc % 2}")
                for t in range(8):
                    nc.tensor.matmul(
                        py,
                        lhsT=wo_t[:, t, mc, :],
                        rhs=ot_rp[:, hc * 2:hc * 2 + 2, :, 2 * t],
                        start=(t == 0),
                        stop=(t == 7),
                    )
                ysb = y_pool.tile([128, 256], F32, name="ysb", tag=f"ysb{mc % 2}")
                nc.vector.tensor_copy(ysb, py)
                nc.sync.dma_start(
                    ap["ypt"][mc * 128:(mc + 1) * 128, hc * 256:(hc + 1) * 256], ysb
                )

            for hc in range(2):
                for qt in range(NST):
                    qs0 = qt * 512
                    u = [
                        ps_u.tile([65, 512], F32, name=f"u{hi}", tag=f"u{hi}")
                        for hi in range(2)
                    ]
                    es = {}
                    for kc in range(NSK + LAG):
                        if kc >= LAG:
                            ka = kc - LAG
                            for hi in range(2):
                                nc.tensor.matmul(
                                    u[hi],
                                    lhsT=vsb[:, ka, hc * 2 + hi, :],
                                    rhs=es[ka][:, hi * 512:(hi + 1) * 512],
                                    start=(ka == 0),
                                    stop=(ka == NSK - 1),
                                )
                            if ka > 0:
                                del es[ka - 1]
                        if kc < NSK:
                            # both heads' scores side by side in one 2-bank group;
                            # full-128 contraction via zero-padded ke (64-deep
                            # f32r matmuls stream at half rate)
                            g = ps_s.tile([128, 1024], F32, tag="sg", name="sg")
                            for hi in range(2):
                                nc.tensor.matmul(
                                    g[:, hi * 512:(hi + 1) * 512],
                                    lhsT=ke_z[hc][kc // 4][:, hi, (kc % 4) * 128:(kc % 4 + 1) * 128],
                                    rhs=qe_t[hc][qt],
                                    start=True,
                                    stop=True,
                                )
                            e = e_pool.tile([128, 1024], MM_DT, name="e", tag="e")
                            nc.scalar.activation(
                                e, g, mybir.ActivationFunctionType.Exp, scale=0.125
                            )
                            es[kc] = e
                    # ---- normalize: ot = U[0:64] * (1/U[64]) ----
                    # denominator rows first so recip/broadcast overlap the
                    # bulk usb copies (custom DVE ops mis-read partition-base-
                    # 64 inputs, so stage them at partition 0)
                    d1s = []
                    for hi in range(2):
                        d1 = rcp_pool.tile([1, 512], F32, tag=f"d{hi}")
                        nc.vector.tensor_copy(d1, u[hi][64:65, :])
                        d1s.append(d1)
                    rbs = []
                    for hi in range(2):
                        r1 = rcp_pool.tile([1, 512], F32, tag=f"r{hi}")
                        nc.vector.reciprocal_approx_fast(r1, d1s[hi])
                        # gpsimd broadcast of 1/denom across 64 partitions
                        dbc = usb_pool.tile([64, 512], F32, tag=f"dbc{hi}")
                        nc.gpsimd.partition_broadcast(dbc, r1)
                        rbs.append(dbc)
                    usbs = []
                    for hi in range(2):
                        usb = usb_pool.tile([64, 512], F32, tag=f"usb{hi}")
                        nc.vector.tensor_copy(usb, u[hi][0:64, :])
                        usbs.append(usb)
                    for hi in range(2):
                        h = hc * 2 + hi
                        nc.vector.tensor_mul(
                            ot64[0:64, h, qs0:qs0 + 512], usbs[hi], rbs[hi]
                        )
                        # shift-by-one dup for phase D's 128-deep contraction;
                        # slot qs0+511 (g=15) is never read by phase D
                        nc.sync.dma_start(
                            ot64[64:128, h, qs0:qs0 + 511],
                            ot64[0:64, h, qs0 + 1:qs0 + 512],
                        )
                    if hc == 1:
                        # fill PE slack in the scalar-paced hc=1 tiles with
                        # the first head pair's output projection
                        emit_d_chunk(0, 2 * qt)
                        emit_d_chunk(0, 2 * qt + 1)
            for mc in range(NKC):
                emit_d_chunk(1, mc)
            if debug:
                nc.sync.dma_start(ap["ot_dbg"], ot64.bitcast(F32))


def _build(debug=False):
    nc = bacc.Bacc("TRN2", target_bir_lowering=False, debug=False, num_devices=N_CORES)
    ap = {}
    ap["xt"] = nc.dram_tensor("xt", [DM, S], F32, kind="ExternalInput").ap()
    ap["wq"] = nc.dram_tensor("wq", [DM, GD], F32, kind="ExternalInput").ap()
    ap["wk"] = nc.dram_tensor("wk", [DM, GD], F32, kind="ExternalInput").ap()
    ap["wv"] = nc.dram_tensor("wv", [DM, GD], F32, kind="ExternalInput").ap()
    ap["wo"] = nc.dram_tensor("wo", [DM, DM], F32, kind="ExternalInput").ap()
    ap["bq2"] = nc.dram_tensor("bq2", [2, 128], F32, kind="ExternalInput").ap()
    ap["bk2"] = nc.dram_tensor("bk2", [2, 128], F32, kind="ExternalInput").ap()
    ap["bv"] = nc.dram_tensor("bv", [GD], F32, kind="ExternalInput").ap()
    ap["cosb"] = nc.dram_tensor("cosb", [128, S], F32, kind="ExternalInput").ap()
    ap["sinb"] = nc.dram_tensor("sinb", [128, S], F32, kind="ExternalInput").ap()
    ap["rot"] = nc.dram_tensor("rot", [128, 128], F32, kind="ExternalInput").ap()
    ap["ones2"] = nc.dram_tensor("ones2", [2, 128], F32, kind="ExternalInput").ap()
    # per-core output: Y^T [1024, 512] (columns = 4 heads x 128 block rows)
    ap["ypt"] = nc.dram_tensor("ypt", [DM, 512], F32, kind="ExternalOutput").ap()
    if debug:
        ap["qe_dbg"] = nc.dram_tensor("qe_dbg", [2, NST, 128, 512], F32, kind="ExternalOutput").ap()
        ap["ke_dbg"] = nc.dram_tensor("ke_dbg", [2, NST, 128, 512], F32, kind="ExternalOutput").ap()
        ap["v_dbg"] = nc.dram_tensor("v_dbg", [128, NSK, 4, 65], F32, kind="ExternalOutput").ap()
        ap["ot_dbg"] = nc.dram_tensor("ot_dbg", [128, 4, S], F32, kind="ExternalOutput").ap()

    with tile.TileContext(nc) as tc:
        _emit(nc, tc, ap, debug=debug)
    nc.compile()
    return nc


_CACHE = {}


def _rope_tables():
    inv_freq = (1.0 / (10000.0 ** (np.arange(0, HD, 2, dtype=np.float32) / HD))).astype(np.float32)
    t = np.arange(S, dtype=np.float32)
    freqs = np.outer(t, inv_freq).astype(np.float32)  # [S, 32]
    emb = np.concatenate([freqs, freqs], axis=-1)  # [S, 64]
    cosT = np.cos(emb).astype(np.float32).T  # [64, S]
    sinT = np.sin(emb).astype(np.float32).T
    cosb = np.ascontiguousarray(np.concatenate([cosT, cosT], axis=0))  # [128, S]
    sinb = np.ascontiguousarray(np.concatenate([sinT, sinT], axis=0))
    return cosb, sinb


def _rot_matrix():
    p64 = np.zeros((HD, HD), dtype=np.float32)
    for i in range(32):
        p64[i, i + 32] = -1.0
        p64[i + 32, i] = 1.0
    p = np.zeros((128, 128), dtype=np.float32)
    p[0:64, 0:64] = p64
    p[64:128, 64:128] = p64
    return np.ascontiguousarray(p.T)  # lhsT = P^T


def kernel(x, Wq, bq, Wk, bk, Wv, bv, Wo, bo):
    x = np.asarray(x, dtype=np.float32)
    Wq, bq = np.asarray(Wq, np.float32), np.asarray(bq, np.float32)
    Wk, bk = np.asarray(Wk, np.float32), np.asarray(bk, np.float32)
    Wv, bv = np.asarray(Wv, np.float32), np.asarray(bv, np.float32)
    Wo, bo = np.asarray(Wo, np.float32), np.asarray(bo, np.float32)

    if "nc" not in _CACHE:
        _CACHE["nc"] = _build()
    nc = _CACHE["nc"]

    cosb, sinb = _rope_tables()
    rot = _rot_matrix()
    ones2 = np.zeros((2, 128), dtype=np.float32)
    ones2[0, 0:64] = 1.0
    ones2[1, 64:128] = 1.0
    xt_b = [np.ascontiguousarray(x[b].T) for b in range(B)]  # [DM, S]
    wo_c = np.ascontiguousarray(Wo)

    in_maps = []
    for c in range(N_CORES):
        b, hg = divmod(c, HG)
        sl = slice(hg * GD, (hg + 1) * GD)
        in_maps.append(
            {
                "xt": xt_b[b],
                "wq": np.ascontiguousarray(Wq[:, sl]),
                "wk": np.ascontiguousarray(Wk[:, sl]),
                "wv": np.ascontiguousarray(Wv[:, sl]),
                "wo": wo_c,
                "bq2": np.ascontiguousarray(bq[sl].reshape(2, 128)),
                "bk2": np.ascontiguousarray(bk[sl].reshape(2, 128)),
                "bv": np.ascontiguousarray(bv[sl]),
                "cosb": cosb,
                "sinb": sinb,
                "rot": rot,
                "ones2": ones2,
            }
        )

    res = bass_utils.run_bass_kernel_spmd(nc, in_maps, core_ids=list(range(N_CORES)))
    _CACHE["last_results"] = res

    # Block placement: core (b, hg), local head hl -> global head h = hg*4+hl,
    # lands at out[h//8, (h%8)*256 + b*128 : +128, :].
    out = np.empty((B, S, DM), dtype=np.float32)
    for c in range(N_CORES):
        b, hg = divmod(c, HG)
        ypt = res.results[c]["ypt"]  # [1024, 512]
        for hl in range(4):
            h = hg * 4 + hl
            b2 = h // 8
            s2 = (h % 8) * 256 + b * 128
            out[b2, s2:s2 + 128, :] = ypt[:, hl * 128:(hl + 1) * 128].T
    out += bo[None, None, :]
    return out
